# revision 2
# baseline (speedup 1.0000x reference)
"""Fused attention kernel for nn_Attention_1090921693811, one sample per core.

pass 1 (q,k):  conv1x1 GEMM (f32r) -> drain to padded bf16 (A + odd-shift B)
               -> depthwise 3x3 via ts-mult + tt-add on DVE/ACT
               -> PE transpose -> Gram accumulation in PSUM (per head)
mid:           norms, temperature, top-k thresholds (max8+match_replace),
               masked-softmax combine -> A -> mhatT = (Wproj@Ablk)^T
pass 2 (v):    conv1x1 GEMM (f32r) -> depthwise 3x3 as diag-stationary
               matmuls accumulated in PSUM -> v_dw (bf16)
               -> out = mhatT.T @ v_dw -> DRAM
"""
import sys
sys.path.insert(0, '/opt/trn_rl_repo')
import numpy as np
import ml_dtypes
from contextlib import ExitStack
from concourse import bass, bacc, mybir, tile

F32 = mybir.dt.float32
F32R = mybir.dt.float32r
BF16 = mybir.dt.bfloat16
Alu = mybir.AluOpType
Act = mybir.ActivationFunctionType

C = 192; C3 = 576; HEADS = 4; CH = 48; H = 128; W = 128; N = H * W
R = 16                   # stripe output rows
NS = H // R              # stripes
SROWS = R + 2            # buffer rows incl halo
STRIDE = 130             # padded row stride: [128 data][2 pad]
ABUF = 2 + (SROWS + 1) * STRIDE  # guard(2) + rows + slack row for rearranges
TOPKS = (24, 32, 36, 38)
NEG = -1e30

# o-tiles: T0=q[0:128] T1=q[128:192]+k[0:64] T2=k[64:192] T3=v[0:128] T4=v[128:192]
OT = [(0, 128), (128, 128), (256, 128), (384, 128), (512, 64)]


def host_prep(x, w_qkv, w_dw, w_proj, temperature, attn1, attn2, attn3, attn4):
    x = np.asarray(x, np.float32).reshape(C, N)
    wq = np.asarray(w_qkv, np.float32).reshape(3 * C, C)     # [o, m]
    wdw = np.asarray(w_dw, np.float32).reshape(3 * C, 9)     # [c, t], t=(dy+1)*3+(dx+1)
    wp = np.asarray(w_proj, np.float32).reshape(C, C)        # [o, c]
    temp = np.asarray(temperature, np.float32).reshape(HEADS)
    wgts = np.stack([np.float32(np.asarray(a).reshape(())) for a in
                     (attn1, attn2, attn3, attn4)])
    d = {}
    d["x"] = x
    d["wqkvT"] = np.ascontiguousarray(wq.T)                  # [192, 576]
    d["wdw"] = np.ascontiguousarray(wdw)
    dg3 = np.zeros((128, 9 * 128), ml_dtypes.bfloat16)
    dg4 = np.zeros((64, 9 * 64), ml_dtypes.bfloat16)
    for t in range(9):
        dg3[np.arange(128), t * 128 + np.arange(128)] = wdw[384:512, t].astype(ml_dtypes.bfloat16)
        dg4[np.arange(64), t * 64 + np.arange(64)] = wdw[512:576, t].astype(ml_dtypes.bfloat16)
    d["diag3"] = dg3
    d["diag4"] = dg4
    d["ident"] = np.eye(128, dtype=ml_dtypes.bfloat16)
    wpt = np.zeros((CH, HEADS * C), ml_dtypes.bfloat16)
    for h in range(HEADS):
        wpt[:, h * C:(h + 1) * C] = wp.T[h * CH:(h + 1) * CH, :].astype(ml_dtypes.bfloat16)
    d["wprojT"] = wpt
    d["temp_rep"] = np.ascontiguousarray(np.broadcast_to(temp[None, :], (CH, HEADS))).astype(np.float32)
    d["wgt_rep"] = np.ascontiguousarray(np.broadcast_to(wgts[None, :], (CH, 4))).astype(np.float32)
    d["ones1"] = np.ones((1, CH), np.float32)
    return d


def build(debug=()):
    nc = bacc.Bacc("TRN2", target_bir_lowering=False)
    E = {}
    for name, shape, dt in [
            ("x", [C, N], F32R), ("wqkvT", [C, C3], F32R), ("wdw", [C3, 9], F32),
            ("diag3", [128, 9 * 128], BF16), ("diag4", [64, 9 * 64], BF16),
            ("ident", [128, 128], BF16), ("wprojT", [CH, HEADS * C], BF16),
            ("temp_rep", [CH, HEADS], F32), ("wgt_rep", [CH, 4], F32),
            ("ones1", [1, CH], F32)]:
        E[name] = nc.declare_dram_parameter(name, shape, dt, isOutput=False)
    out_ext = nc.declare_dram_parameter("out", [C, N], F32, isOutput=True)
    dbg_ext = {name: nc.declare_dram_parameter("dbg_" + name, list(shape), F32, isOutput=True)
               for name, shape in debug}
    dbg = dict(debug)

    with ExitStack() as ctx, tile.TileContext(nc) as tc:
        persist = ctx.enter_context(tc.tile_pool(name="persist", bufs=1))
        wqkvT = [persist.tile([128, C3], F32R, tag="wq0"), persist.tile([64, C3], F32R, tag="wq1")]
        nc.sync.dma_start(wqkvT[0][:], E["wqkvT"][0:128, :])
        nc.sync.dma_start(wqkvT[1][:], E["wqkvT"][128:192, :])
        wdw_sb = []
        for i, (o0, ow) in enumerate(OT):
            t_ = persist.tile([ow, 9], F32, tag=f"wdw{i}")
            nc.sync.dma_start(t_[:], E["wdw"][o0:o0 + ow, :])
            wdw_sb.append(t_)
        diag3 = persist.tile([128, 9 * 128], BF16, tag="dg3")
        diag4 = persist.tile([64, 9 * 64], BF16, tag="dg4")
        ident = persist.tile([128, 128], BF16, tag="id")
        wprojT = persist.tile([CH, HEADS * C], BF16, tag="wpt")
        temp_rep = persist.tile([CH, HEADS], F32, tag="tmp_r")
        wgt_rep = persist.tile([CH, 4], F32, tag="wgt_r")
        ones1 = persist.tile([1, CH], F32, tag="on1")
        for t_, name in ((diag3, "diag3"), (diag4, "diag4"), (ident, "ident"),
                         (wprojT, "wprojT"), (temp_rep, "temp_rep"),
                         (wgt_rep, "wgt_rep"), (ones1, "ones1")):
            nc.sync.dma_start(t_[:], E[name][:])

        sumsq = persist.tile([128, 3 * NS], F32, tag="ssq")
        v_dw = [persist.tile([128, N], BF16, tag="vdw0"), persist.tile([64, N], BF16, tag="vdw1")]
        mid = ctx.enter_context(tc.tile_pool(name="mid", bufs=1))

        def load_x_stripe(s, pool):
            r0 = max(s * R - 1, 0)
            r1 = min(s * R + R + 1, H)
            br0 = r0 - (s * R - 1)
            nr = r1 - r0
            ncols = SROWS * W
            xa = pool.tile([128, ncols], F32R, tag="xa")
            xb = pool.tile([64, ncols], F32R, tag="xb")
            if br0 > 0:
                nc.gpsimd.memset(xa[:, 0:W], 0.0)
                nc.gpsimd.memset(xb[:, 0:W], 0.0)
            if br0 + nr < SROWS:
                nc.gpsimd.memset(xa[:, (SROWS - 1) * W:], 0.0)
                nc.gpsimd.memset(xb[:, (SROWS - 1) * W:], 0.0)
            nc.sync.dma_start(xa[:, br0 * W:(br0 + nr) * W], E["x"][0:128, r0 * W:r1 * W])
            nc.sync.dma_start(xb[:, br0 * W:(br0 + nr) * W], E["x"][128:192, r0 * W:r1 * W])
            return xa, xb

        def gemm_tile(gps, xa, xb, o0, ow, drain):
            """conv1x1 for one o-tile over the stripe; drain(psum_ap, rr, nrow) per 1024-group."""
            ncols = SROWS * W
            for g0 in range(0, ncols, 1024):
                gw = min(1024, ncols - g0)
                pg = gps.tile([128, 1024], F32, tag="g")
                for c0 in range(0, gw, 512):
                    cw = min(512, gw - c0)
                    for mi, (mt, xs) in enumerate(((wqkvT[0], xa), (wqkvT[1], xb))):
                        nc.tensor.matmul(pg[:, c0:c0 + cw], mt[:, o0:o0 + ow],
                                         xs[:, g0 + c0:g0 + c0 + cw],
                                         start=(mi == 0), stop=(mi == 1))
                drain(pg, g0 // W, gw // W)

        # =========================== PASS 1 (q, k) ===========================
        with tc.tile_pool(name="p1x", bufs=2) as xp, \
             tc.tile_pool(name="p1ps", bufs=2, space="PSUM") as gps, \
             tc.tile_pool(name="p1ab", bufs=2) as abp, \
             tc.tile_pool(name="p1dw", bufs=2) as dwp, \
             tc.tile_pool(name="p1t", bufs=2) as tp, \
             tc.tile_pool(name="p1tps", bufs=2, space="PSUM") as tps, \
             tc.tile_pool(name="gramp", bufs=1, space="PSUM") as gram_pool:
            gram_ps = gram_pool.tile([CH, HEADS * CH], F32)
            for s in range(NS):
                xa, xb = load_x_stripe(s, xp)
                qkT = tp.tile([128, R * 384], BF16, tag="qkT")
                abufs = []
                for i in range(3):
                    o0, ow = OT[i]
                    A = abp.tile([128, ABUF], BF16, tag=f"A{i}")
                    B = abp.tile([128, ABUF], BF16, tag=f"B{i}")
                    nc.gpsimd.memset(A[:, 0:2], 0.0)
                    nc.gpsimd.memset(A[:, 2:2 + SROWS * STRIDE].rearrange(
                        "p (r c) -> p r c", c=STRIDE)[:, :, 128:130], 0.0)
                    nc.gpsimd.memset(B[:, 0:2], 0.0)
                    nc.gpsimd.memset(B[:, 2:2 + SROWS * STRIDE].rearrange(
                        "p (r c) -> p r c", c=STRIDE)[:, :, 125:130], 0.0)

                    def drain(pg, rr, nrow, A=A, B=B, ow=ow):
                        srcP = pg[:ow, 0:nrow * W].rearrange("p (r c) -> p r c", c=W)
                        dstA = A[:ow, 2 + rr * STRIDE:2 + (rr + nrow) * STRIDE].rearrange(
                            "p (r c) -> p r c", c=STRIDE)[:, :, 0:128]
                        nc.scalar.copy(dstA, srcP)
                        dstB = B[:ow, 1 + rr * STRIDE:1 + (rr + nrow) * STRIDE].rearrange(
                            "p (r c) -> p r c", c=STRIDE)[:, :, 0:128]
                        nc.vector.tensor_copy(dstB, srcP)
                    gemm_tile(gps, xa, xb, o0, ow, drain)
                    abufs.append((A, B))

                for i in range(3):
                    o0, ow = OT[i]
                    A, B = abufs[i]
                    w9 = wdw_sb[i]
                    acc = dwp.tile([128, R * W], BF16, tag=f"acc{i}")

                    def src_ap(buf, base, ow=ow):
                        return buf[:ow, base:base + R * STRIDE].rearrange(
                            "p (r c) -> p r c", c=STRIDE)[:, :, 0:128]
                    acc3 = acc[:ow].rearrange("p (r c) -> p r c", c=W)
                    nc.vector.tensor_scalar(acc3, src_ap(A, 2 + STRIDE), w9[:ow, 4:5], None, Alu.mult)
                    ntmp = 0
                    for dy in (-1, 0, 1):
                        for dx in (-1, 0, 1):
                            if dy == 0 and dx == 0:
                                continue
                            t = (dy + 1) * 3 + (dx + 1)
                            if dx == 0:
                                sap = src_ap(A, 2 + (1 + dy) * STRIDE)
                            else:
                                sap = src_ap(B, 2 + (1 + dy) * STRIDE + dx - 1)
                            tmp = dwp.tile([128, R * W], BF16, tag=f"tmp{ntmp % 2}")
                            ntmp += 1
                            dap = tmp[:ow].rearrange("p (r c) -> p r c", c=W)
                            if t in (0, 6):
                                nc.scalar.activation(dap, sap, Act.Copy, bias=0.0, scale=w9[:ow, t:t + 1])
                            else:
                                nc.vector.tensor_scalar(dap, sap, w9[:ow, t:t + 1], None, Alu.mult)
                            nc.vector.tensor_tensor(acc[:ow], acc[:ow], tmp[:ow], Alu.add)
                    sq = dwp.tile([128, R * W], BF16, tag="sq")
                    nc.scalar.activation(sq[:ow], acc[:ow], Act.Square,
                                         accum_out=sumsq[:ow, i * NS + s:i * NS + s + 1])
                    for rc in range(R):
                        pt = tps.tile([128, 128], F32, tag="t")
                        nc.tensor.transpose(pt[:, 0:ow], acc[:ow, rc * 128:(rc + 1) * 128], ident[:])
                        nc.scalar.copy(qkT[:, rc * 384 + i * 128: rc * 384 + i * 128 + ow], pt[:, 0:ow])
                for rc in range(R):
                    for h in range(HEADS):
                        nc.tensor.matmul(gram_ps[:, h * CH:(h + 1) * CH],
                                         qkT[:, rc * 384 + h * CH: rc * 384 + (h + 1) * CH],
                                         qkT[:, rc * 384 + 192 + h * CH: rc * 384 + 192 + (h + 1) * CH],
                                         start=(s == 0 and rc == 0), stop=(s == NS - 1 and rc == R - 1),
                                         skip_group_check=True)

            # ======================= MID: softmax etc =======================
            ssq_col = mid.tile([128, 3], F32, tag="ssqc")
            for i in range(3):
                nc.vector.tensor_reduce(ssq_col[:, i:i + 1], sumsq[:, i * NS:(i + 1) * NS], Alu.add)
            rqk = mid.tile([CH, 8], F32, tag="rqk")
            for j in range(8):
                g = j * CH if j < 4 else 192 + (j - 4) * CH
                i, p = divmod(g, 128)
                if p + CH <= 128:
                    nc.sync.dma_start(rqk[:, j:j + 1], ssq_col[p:p + CH, i:i + 1])
                else:
                    k1 = 128 - p
                    nc.sync.dma_start(rqk[0:k1, j:j + 1], ssq_col[p:128, i:i + 1])
                    nc.sync.dma_start(rqk[k1:CH, j:j + 1], ssq_col[0:CH - k1, i + 1:i + 2])
            rqk2 = mid.tile([CH, 8], F32, tag="rqk2")
            nc.scalar.sqrt(rqk2[:], rqk[:])
            nc.vector.reciprocal(rqk[:], rqk2[:])
            rk_row = mid.tile([4, CH], F32, tag="rkrow")
            for h in range(HEADS):
                nc.sync.dma_start(rk_row[h:h + 1, :], rqk[:, 4 + h:5 + h])
            with tc.tile_pool(name="midps", bufs=1, space="PSUM") as mps:
                rk_rep_ps = mps.tile([CH, HEADS * CH], F32, tag="rkrep")
                for h in range(HEADS):
                    nc.tensor.matmul(rk_rep_ps[:, h * CH:(h + 1) * CH], ones1[:],
                                     rk_row[h:h + 1, :], start=True, stop=True)
                attn = mid.tile([CH, HEADS * CH], F32, tag="attn")
                nc.vector.tensor_tensor(attn[:], gram_ps[:], rk_rep_ps[:], Alu.mult)
                if "gram" in dbg:
                    g_sb = mid.tile([CH, HEADS * CH], F32, tag="gsb")
                    nc.vector.tensor_copy(g_sb[:], gram_ps[:])
                    nc.sync.dma_start(dbg_ext["gram"][:], g_sb[:])
                s_col = mid.tile([CH, HEADS], F32, tag="scol")
                nc.vector.tensor_tensor(s_col[:], rqk[:, 0:4], temp_rep[:], Alu.mult)
                srt = mid.tile([CH, 5 * 8], F32, tag="srt")
                scratch = mid.tile([CH, HEADS * CH], F32, tag="scr")
                e_t = mid.tile([CH, HEADS * CH], F32, tag="e")
                acc_m = mid.tile([CH, HEADS * CH], F32, tag="accm")
                mx = mid.tile([CH, 8], F32, tag="mx")
                sk = mid.tile([CH, 4], F32, tag="sk")
                cf = mid.tile([CH, 4], F32, tag="cf")
                junk = mid.tile([CH, CH], F32, tag="junk")
                for h in range(HEADS):
                    ah = attn[:, h * CH:(h + 1) * CH]
                    sc = scratch[:, h * CH:(h + 1) * CH]
                    nc.vector.tensor_copy(sc, ah)
                    for it in range(5):
                        nc.vector.max(srt[:, it * 8:(it + 1) * 8], sc)
                        if it < 4:
                            nc.vector.match_replace(sc, srt[:, it * 8:(it + 1) * 8], sc, NEG)
                    nc.vector.tensor_scalar(mx[:, h:h + 1], srt[:, 0:1], s_col[:, h:h + 1],
                                            -1.0, Alu.mult, Alu.mult)
                    eh = e_t[:, h * CH:(h + 1) * CH]
                    nc.scalar.activation(eh, ah, Act.Exp, bias=mx[:, h:h + 1], scale=s_col[:, h:h + 1])
                    for ki, kk in enumerate(TOPKS):
                        th = srt[:, kk - 1:kk]
                        nc.vector.scalar_tensor_tensor(junk[:], ah, th, eh, Alu.is_ge, Alu.mult,
                                                       accum_out=sk[:, ki:ki + 1])
                    nc.vector.reciprocal(sk[:], sk[:])
                    nc.vector.tensor_tensor(cf[:], sk[:], wgt_rep[:], Alu.mult)
                    am = acc_m[:, h * CH:(h + 1) * CH]
                    for ki, kk in enumerate(TOPKS):
                        th = srt[:, kk - 1:kk]
                        if ki == 0:
                            nc.vector.tensor_scalar(am, ah, th, cf[:, ki:ki + 1], Alu.is_ge, Alu.mult)
                        else:
                            nc.vector.tensor_scalar(junk[:], ah, th, cf[:, ki:ki + 1], Alu.is_ge, Alu.mult)
                            nc.vector.tensor_tensor(am, am, junk[:], Alu.add)
                    nc.vector.tensor_tensor(am, am, eh, Alu.mult)
                a_bf = mid.tile([CH, HEADS * CH], BF16, tag="abf")
                nc.vector.tensor_copy(a_bf[:], acc_m[:])
                mh_sb = mid.tile([CH, HEADS * C], BF16, tag="mhsb")
                mhat_ps = [mps.tile([CH, 2 * C], F32, tag=f"mh{j}") for j in range(2)]
                for h in range(HEADS):
                    nc.tensor.matmul(mhat_ps[h // 2][:, (h % 2) * C:(h % 2 + 1) * C],
                                     a_bf[:, h * CH:(h + 1) * CH], wprojT[:, h * C:(h + 1) * C],
                                     start=True, stop=True)
                    nc.vector.tensor_copy(mh_sb[:, h * C:(h + 1) * C],
                                          mhat_ps[h // 2][:, (h % 2) * C:(h % 2 + 1) * C])
                if "attn" in dbg:
                    nc.sync.dma_start(dbg_ext["attn"][:], attn[:])
                if "accm" in dbg:
                    nc.sync.dma_start(dbg_ext["accm"][:], acc_m[:])
                if "rqk" in dbg:
                    nc.sync.dma_start(dbg_ext["rqk"][:], rqk[:])
            mhatT = [mid.tile([128, C], BF16, tag="mhs0"), mid.tile([64, C], BF16, tag="mhs1")]
            for h in range(HEADS):
                p0 = h * CH
                if p0 + CH <= 128:
                    nc.sync.dma_start(mhatT[0][p0:p0 + CH, :], mh_sb[:, h * C:(h + 1) * C])
                elif p0 >= 128:
                    nc.sync.dma_start(mhatT[1][p0 - 128:p0 - 128 + CH, :], mh_sb[:, h * C:(h + 1) * C])
                else:
                    k1 = 128 - p0
                    nc.sync.dma_start(mhatT[0][p0:128, :], mh_sb[0:k1, h * C:(h + 1) * C])
                    nc.sync.dma_start(mhatT[1][0:CH - k1, :], mh_sb[k1:CH, h * C:(h + 1) * C])

        # =========================== PASS 2 (v) ===========================
        with tc.tile_pool(name="p2x", bufs=2) as xp2, \
             tc.tile_pool(name="p2ps", bufs=2, space="PSUM") as gps2, \
             tc.tile_pool(name="p2ab", bufs=2) as abp2, \
             tc.tile_pool(name="p2dps", bufs=3, space="PSUM") as dps2:
            for s in range(NS):
                xa, xb = load_x_stripe(s, xp2)
                for vi, i in enumerate((3, 4)):
                    o0, ow = OT[i]
                    A = abp2.tile([ow, ABUF], BF16, tag=f"VA{vi}")
                    nc.gpsimd.memset(A[:, 0:2], 0.0)
                    nc.gpsimd.memset(A[:, 2:2 + SROWS * STRIDE].rearrange(
                        "p (r c) -> p r c", c=STRIDE)[:, :, 128:130], 0.0)

                    def drainv(pg, rr, nrow, A=A, ow=ow):
                        dstA = A[:ow, 2 + rr * STRIDE:2 + (rr + nrow) * STRIDE].rearrange(
                            "p (r c) -> p r c", c=STRIDE)[:, :, 0:128]
                        nc.scalar.copy(dstA, pg[:ow, 0:nrow * W].rearrange("p (r c) -> p r c", c=W))
                    gemm_tile(gps2, xa, xb, o0, ow, drainv)
                    dgt = diag3 if ow == 128 else diag4
                    for ch0 in range(0, R, 4):
                        pv = dps2.tile([ow, 512], F32, tag=f"v{vi}")
                        first = True
                        for dy in (-1, 0, 1):
                            for dx in (-1, 0, 1):
                                t = (dy + 1) * 3 + (dx + 1)
                                base = 2 + (1 + ch0 + dy) * STRIDE + dx
                                mov = A[:ow, base:base + 4 * STRIDE].rearrange(
                                    "p (r c) -> p r c", c=STRIDE)[:, :, 0:128]
                                nc.tensor.matmul(pv[:ow, :].rearrange("p (r c) -> p r c", c=W),
                                                 dgt[:, t * ow:(t + 1) * ow], mov,
                                                 start=first, stop=(t == 8), skip_group_check=True)
                                first = False
                        nc.scalar.copy(v_dw[vi][:ow, (s * R + ch0) * W:(s * R + ch0 + 4) * W], pv[:ow, :])
        if "vdw0" in dbg:
            vv = mid.tile([128, N], F32, tag="vv")
            nc.vector.tensor_copy(vv[:], v_dw[0][:])
            nc.sync.dma_start(dbg_ext["vdw0"][:], vv[:])
        with tc.tile_pool(name="p2o", bufs=3) as op, \
             tc.tile_pool(name="p2ops", bufs=4, space="PSUM") as ops_:
            for oo0, oow in ((0, 128), (128, 64)):
                for n0 in range(0, N, 512):
                    po = ops_.tile([128, 512], F32, tag="o")
                    for mi2 in range(2):
                        nc.tensor.matmul(po[:oow, :], mhatT[mi2][:, oo0:oo0 + oow],
                                         v_dw[mi2][:, n0:n0 + 512],
                                         start=(mi2 == 0), stop=(mi2 == 1))
                    ot = op.tile([128, 512], F32, tag="ot")
                    nc.vector.tensor_copy(ot[:oow, :], po[:oow, :])
                    nc.sync.dma_start(out_ext[oo0:oo0 + oow, n0:n0 + 512], ot[:oow, :])
    nc.finalize()
    return nc


from concourse.bass_utils import run_bass_kernel_spmd

B = 8
_CACHE = {}


def kernel(**inputs):
    """Full (unsharded) inputs -> full output [8, 192, 128, 128] float32.

    Shards batch across 8 NeuronCores (one sample each), runs the fused
    Bass kernel SPMD, gathers results.
    """
    x = np.asarray(inputs["x"], np.float32)
    if "nc" not in _CACHE:
        _CACHE["nc"] = build()
    nc = _CACHE["nc"]
    in_maps = [host_prep(x[b], inputs["w_qkv"], inputs["w_dw"], inputs["w_proj"],
                         inputs["temperature"], inputs["attn1"], inputs["attn2"],
                         inputs["attn3"], inputs["attn4"]) for b in range(B)]
    res = run_bass_kernel_spmd(nc, in_maps, list(range(B)))
    out = np.stack([res.results[b]["out"].reshape(C, H, W) for b in range(B)])
    return out.astype(np.float32)
